# revision 51
# baseline (speedup 1.0000x reference)
"""Trainium2 Bass kernel for nn_AttnBlock (B=16, C=512, H=W=32, T=180, G=32).

Math: the module broadcasts the text condition across channels, so k/v rows are
identical for every channel and the whole attention block collapses to rank-1:

  per batch b:
    group-norm stats over x[b]:   mu_g, rstd_g (32 groups of 16 ch x 1024 pix)
    wqg[c] = (sum_o wq[o,c]) * gamma[c];  a[c] = wqg[c]*rstd_{g(c)}
    s'[n]  = sum_c a[c]*x[c,n] + const_b
    kb[f]  = SCALE*(wk @ cond_b + bk) ;  vb[f] = wv @ cond_b + bv
    w[n]   = (sum_f vb[f]*e^{kb[f] s'[n]}) / (sum_f e^{kb[f] s'[n]})
    out[c,n] = x[c,n] + wo_rowsum[c]*w[n] + bo[c]

and since |kb*s'| stays small, w(s') is replaced by its cubic Taylor series
w ~= c0 + c1 s' + c2 s'^2 + c3 s'^3 whose coefficients come from the moment
sums T_j = sum_f vb kb^j/j!, U_j = sum_f kb^j/j! (truncation error ~2e-4,
well under the 2e-2 gate; bf16 staging dominates at ~6e-3).

Sharding: data-parallel over batch, 2 batches per core, 8 cores, no
collectives. x/out/cond ride HBM as bf16 (halves DMA, 2x DVE modes); k/v
matvecs and all moment/broadcast reductions run on PE against
host-transposed bf16 weights; weight-derived scalars are host-folded.
PSUM: 4 s-accumulator banks + 4 small-ring banks. Engine APs must start at
partition 0/32/64/96 - all row intermediates ([1,N] tiles) sit at p0.
"""
import numpy as np
from contextlib import ExitStack

B, C, HW, N, T = 16, 512, 32, 1024, 180
F = 1024                      # in_features == H*W
G = 32                        # groups; 16 channels per group
NCORES, BPC = 8, 2            # cores, batches per core
NCH = C // 128                # 4 channel chunks
NFC = F // 128                # 8 feature chunks
EPS = 1e-6
SCALE = float(C) ** -0.5

_CACHE = {}


def _legalize_sync(nc, mybir):
    """This walrus build accepts at most one sync-wait command per
    instruction; hoist extra waits onto preceding same-engine NOPs."""
    k = 0
    for fn in nc.m.functions:
        for blk in fn.blocks:
            new = []
            for ins in blk.instructions:
                si = ins.sync_info
                if si is not None and si.on_wait is not None and len(si.on_wait) > 1:
                    for w in list(si.on_wait[:-1]):
                        nop = mybir.InstNoOp(name=f"syncsplit-{k}", ins=[], outs=[])
                        k += 1
                        nop.engine = ins.engine
                        nop.sync_info = mybir.SyncInfo(on_wait=[w], on_update=[])
                        new.append(nop)
                    ins.sync_info = mybir.SyncInfo(
                        on_wait=[si.on_wait[-1]],
                        on_update=list(si.on_update or []))
                new.append(ins)
            blk.instructions[:] = new


def _build(reps=1, legalize=True, store_out=True, load_x=True, xbufs=4):
    import concourse.bass as bass
    import concourse.mybir as mybir
    import concourse.tile as tile

    f32 = mybir.dt.float32
    bf16 = mybir.dt.bfloat16
    Act = mybir.ActivationFunctionType
    Alu = mybir.AluOpType

    nc = bass.Bass()

    x_d = nc.dram_tensor("x_sh", [BPC, C, N], bf16, kind="ExternalInput")
    cond_d = nc.dram_tensor("cond_sh", [BPC, T], bf16, kind="ExternalInput")
    wqg_d = nc.dram_tensor("wqg", [C], f32, kind="ExternalInput")
    wors_d = nc.dram_tensor("wo_rs", [C], f32, kind="ExternalInput")
    bo_d = nc.dram_tensor("bo", [C], f32, kind="ExternalInput")
    bks_d = nc.dram_tensor("bks", [F], f32, kind="ExternalInput")
    bv_d = nc.dram_tensor("bv", [F], f32, kind="ExternalInput")
    wqb_d = nc.dram_tensor("wqb", [1], f32, kind="ExternalInput")
    wkT_d = nc.dram_tensor("wkT", [T, F], bf16, kind="ExternalInput")
    wvT_d = nc.dram_tensor("wvT", [T, F], bf16, kind="ExternalInput")
    ind128_d = nc.dram_tensor("ind128", [128, 8], f32, kind="ExternalInput")
    eye_d = nc.dram_tensor("eye128", [128, 128], bf16, kind="ExternalInput")
    indT8_d = nc.dram_tensor("indT8", [8, 128], f32, kind="ExternalInput")
    out_d = nc.dram_tensor("out", [BPC, C, N], bf16, kind="ExternalOutput")

    with tile.TileContext(nc) as tc, ExitStack() as ctx:
        singles = ctx.enter_context(tc.tile_pool(name="singles", bufs=1))
        xpool = ctx.enter_context(tc.tile_pool(name="xpool", bufs=xbufs))
        ypool = ctx.enter_context(tc.tile_pool(name="ypool", bufs=4))
        opool = ctx.enter_context(tc.tile_pool(name="opool", bufs=3))
        bpool = ctx.enter_context(tc.tile_pool(name="bpool", bufs=2))
        ps_sm = ctx.enter_context(tc.tile_pool(name="ps_sm", bufs=8, space="PSUM"))

        # constants + ACT exp-table preload first (ACT ring is in-order)
        ones_col = singles.tile([128, 1], f32)
        nc.vector.memset(ones_col, 1.0)
        ones_row_b = singles.tile([1, 128], bf16)
        nc.vector.memset(ones_row_b, 1.0)
        ones_row_f = singles.tile([1, 128], f32)
        nc.vector.memset(ones_row_f, 1.0)
        ones_row_n = singles.tile([1, N], bf16)
        nc.vector.memset(ones_row_n, 1.0)
        factrow = singles.tile([1, 8], f32)
        nc.vector.memset(factrow, 1.0)
        nc.vector.memset(factrow[0:1, 2:3], 0.5)
        nc.vector.memset(factrow[0:1, 3:4], 1.0 / 6.0)
        nc.vector.memset(factrow[0:1, 6:7], 0.5)
        nc.vector.memset(factrow[0:1, 7:8], 1.0 / 6.0)
        eps8 = singles.tile([8, 1], f32)
        nc.vector.memset(eps8, EPS)
        tl = singles.tile([1, 1], f32)
        nc.scalar.activation(tl, eps8[0:1, 0:1], Act.Exp)

        # ---------------- prologue loads (ACT ring; idle during body) -------
        xts, condTs = [], []
        xt0 = xpool.tile([128, NCH, N], bf16, tag="xt", name="xt0")
        nc.sync.dma_start(xt0, x_d[0].rearrange("(a p) n -> p a n", p=128))
        xt1 = xpool.tile([128, NCH, N], bf16, tag="xt", name="xt1")
        nc.sync.dma_start(xt1, x_d[1].rearrange("(a p) n -> p a n", p=128))
        xts.append(xt0)
        xts.append(xt1)
        for b in range(BPC):
            cta = bpool.tile([128, 1], bf16, tag="cta", name=f"cta{b}")
            nc.sync.dma_start(cta, cond_d[b, 0:128].rearrange("(p o) -> p o", o=1))
            ctb = bpool.tile([52, 1], bf16, tag="ctb", name=f"ctb{b}")
            nc.sync.dma_start(ctb, cond_d[b, 128:180].rearrange("(p o) -> p o", o=1))
            condTs.append((cta, ctb))
        wkTa = singles.tile([128, F], bf16)
        nc.scalar.dma_start(wkTa, wkT_d[0:128, :])
        wkTb = singles.tile([52, F], bf16)
        nc.scalar.dma_start(wkTb, wkT_d[128:180, :])
        wvTa = singles.tile([128, F], bf16)
        nc.scalar.dma_start(wvTa, wvT_d[0:128, :])
        wvTb = singles.tile([52, F], bf16)
        nc.scalar.dma_start(wvTb, wvT_d[128:180, :])
        ind128 = singles.tile([128, 8], f32)
        nc.scalar.dma_start(ind128, ind128_d[:, :])
        indT8 = singles.tile([8, 128], f32)
        nc.scalar.dma_start(indT8, indT8_d[:, :])
        wqg_pc = singles.tile([128, NCH], f32)
        nc.scalar.dma_start(wqg_pc, wqg_d[:].rearrange("(a p) -> p a", p=128))
        wors_pc = singles.tile([128, NCH], f32)
        nc.scalar.dma_start(wors_pc, wors_d[:].rearrange("(a p) -> p a", p=128))
        bo_pc = singles.tile([128, NCH], f32)
        nc.scalar.dma_start(bo_pc, bo_d[:].rearrange("(a p) -> p a", p=128))
        bks_pc = singles.tile([128, NFC], f32)
        nc.scalar.dma_start(bks_pc, bks_d[:].rearrange("(a p) -> p a", p=128))
        bv_pc = singles.tile([128, NFC], f32)
        nc.scalar.dma_start(bv_pc, bv_d[:].rearrange("(a p) -> p a", p=128))
        wqb_sb = singles.tile([1, 1], f32)
        nc.scalar.dma_start(wqb_sb, wqb_d[:].rearrange("(p o) -> p o", o=1))
        eye128 = singles.tile([128, 128], bf16)
        nc.scalar.dma_start(eye128, eye_d[:, :])

        # ---------------- per-batch stages (software-pipelined emission) ----
        # state keyed by (batch, rep parity) so rep r+1's front half can be
        # emitted while rep r's tail still reads its own tiles
        S = {}

        def stage_load(b, rep_i):
            k = (b, rep_i & 1)
            if rep_i == 0 or not load_x:
                if rep_i == 0:
                    S[k] = {"xt": xts[b], "condT": condTs[b]}
                else:
                    S[k] = {"xt": S[(b, (rep_i - 1) & 1)]["xt"],
                            "condT": S[(b, (rep_i - 1) & 1)]["condT"]}
            else:
                xt = xpool.tile([128, NCH, N], bf16, tag="xt", name=f"xtr{b}")
                nc.sync.dma_start(xt, x_d[b].rearrange("(a p) n -> p a n", p=128))
                cta = bpool.tile([128, 1], bf16, tag="cta", name=f"ctar{b}")
                nc.scalar.dma_start(cta,
                                    cond_d[b, 0:128].rearrange("(p o) -> p o", o=1))
                ctb = bpool.tile([52, 1], bf16, tag="ctb", name=f"ctbr{b}")
                nc.scalar.dma_start(ctb,
                                    cond_d[b, 128:180].rearrange("(p o) -> p o", o=1))
                S[k] = {"xt": xt, "condT": (cta, ctb)}

        def stage_stats_a(k):
            xt = S[k]["xt"]
            mv2 = bpool.tile([128, NCH, 2], f32, tag="mv2", name=f"mv2_{k[0]}")
            mv = bpool.tile([128, NCH, 2], f32, tag="mv", name=f"mv_{k[0]}")
            for ch in range(NCH):
                st = bpool.tile([128, 2, 6], f32, tag="st", name=f"st{k[0]}{ch}")
                nc.vector.bn_stats(st[:, 0, :], xt[:, ch, 0:512])
                nc.vector.bn_stats(st[:, 1, :], xt[:, ch, 512:1024])
                nc.vector.bn_aggr(mv[:, ch, :], st)
            msq = bpool.tile([128, NCH], f32, tag="msq", name=f"msq{k[0]}")
            nc.vector.tensor_mul(msq, mv[:, :, 0], mv[:, :, 0])
            nc.vector.tensor_copy(mv2[:, :, 0], mv[:, :, 0])
            nc.vector.tensor_add(mv2[:, :, 1], mv[:, :, 1], msq)
            S[k]["mv2"] = mv2

        def stage_stats_b(k):
            mv2 = S[k]["mv2"]
            gstat_ps = ps_sm.tile([8, NCH, 2], f32, tag="sm", name=f"gst{k[0]}")
            for ch in range(NCH):
                nc.tensor.matmul(gstat_ps[:, ch, :], ind128, mv2[:, ch, :],
                                 start=True, stop=True)
            gsb = bpool.tile([8, NCH, 2], f32, tag="gsb", name=f"gsb{k[0]}")
            nc.vector.tensor_copy(gsb, gstat_ps)
            msqg = bpool.tile([8, NCH], f32, tag="msqg", name=f"msqg{k[0]}")
            nc.vector.tensor_mul(msqg, gsb[:, :, 0], gsb[:, :, 0])
            varg = bpool.tile([8, NCH], f32, tag="varg", name=f"varg{k[0]}")
            nc.vector.tensor_sub(varg, gsb[:, :, 1], msqg)
            lnv = bpool.tile([8, NCH], f32, tag="lnv", name=f"lnv{k[0]}")
            nc.scalar.activation(lnv, varg, Act.Ln, bias=eps8[:, 0:1])
            rm = bpool.tile([8, 2, NCH], f32, tag="rm", name=f"rm{k[0]}")
            nc.scalar.activation(rm[:, 0, :], lnv, Act.Exp, scale=-0.5)
            nc.vector.tensor_mul(rm[:, 1, :], gsb[:, :, 0], rm[:, 0, :])
            rep_ps = ps_sm.tile([128, 2 * NCH], f32, tag="sm", name=f"rep{k[0]}")
            nc.tensor.matmul(rep_ps, indT8, rm.rearrange("g a c -> g (a c)"),
                             start=True, stop=True)
            rep3 = rep_ps.rearrange("p (a c) -> p a c", a=2)
            a_all = bpool.tile([128, NCH], bf16, tag="a_all", name=f"a_all{k[0]}")
            nc.vector.tensor_mul(a_all, wqg_pc, rep3[:, 0, :])
            wm_all = bpool.tile([128, NCH], f32, tag="wm_all", name=f"wm{k[0]}")
            nc.vector.tensor_mul(wm_all, wqg_pc, rep3[:, 1, :])
            S[k]["a_all"], S[k]["wm_all"] = a_all, wm_all

        def stage_smv_mm(k):
            """s in partition-major layout: sp[p, nb] = s[128*nb + p]."""
            a_all, wm_all, xt = S[k]["a_all"], S[k]["wm_all"], S[k]["xt"]
            sp_ps = ps_sm.tile([128, 8], f32, tag="sm", name=f"sp{k[0]}")
            wms_ps = ps_sm.tile([1, 1], f32, tag="sm", name=f"wms{k[0]}")
            for nb in range(8):
                for ch in range(NCH):
                    nc.tensor.matmul(
                        sp_ps[:, nb:nb + 1],
                        xt[:, ch, 128 * nb:128 * (nb + 1)],
                        a_all[:, ch:ch + 1],
                        start=(ch == 0), stop=(ch == NCH - 1),
                        skip_group_check=True)
            for ch in range(NCH):
                nc.tensor.matmul(wms_ps, wm_all[:, ch:ch + 1], ones_col,
                                 start=(ch == 0), stop=(ch == NCH - 1))
            S[k]["sp_ps"], S[k]["wms_ps"] = sp_ps, wms_ps

        def stage_s(k):
            """partition-major powers sp^1..sp^3, each [128, 8]."""
            sp_ps, wms_ps = S[k]["sp_ps"], S[k]["wms_ps"]
            constb = bpool.tile([1, 1], f32, tag="constb", name=f"cb{k[0]}")
            nc.vector.tensor_sub(constb, wqb_sb, wms_ps)
            cb_ps = ps_sm.tile([128, 1], f32, tag="sm", name=f"cbb{k[0]}")
            nc.tensor.matmul(cb_ps, ones_row_f, constb, start=True, stop=True)
            cb_sb = bpool.tile([128, 1], f32, tag="cb_sb", name=f"cbs{k[0]}")
            nc.vector.tensor_copy(cb_sb, cb_ps)
            s1p = bpool.tile([128, 8], bf16, tag="s1p", name=f"s1p{k[0]}")
            with nc.allow_low_precision(reason="s bf16; 2e-2 budget"):
                nc.vector.tensor_scalar(s1p, sp_ps, cb_sb, None, op0=Alu.add)
            s2p = bpool.tile([128, 8], bf16, tag="s2p", name=f"s2p{k[0]}")
            nc.vector.tensor_mul(s2p, s1p, s1p)
            s3p = bpool.tile([128, 8], bf16, tag="s3p", name=f"s3p{k[0]}")
            nc.vector.tensor_mul(s3p, s2p, s1p)
            S[k]["spows"] = (s1p, s2p, s3p)

        def stage_kv(k):
            cta, ctb = S[k]["condT"]
            kv_ps = ps_sm.tile([128, 2 * NFC], f32, tag="sm", name=f"kv{k[0]}")
            for fc in range(NFC):
                nc.tensor.matmul(kv_ps[:, fc:fc + 1],
                                 wkTa[:, 128 * fc:128 * (fc + 1)], cta,
                                 start=True, stop=False, skip_group_check=True)
                nc.tensor.matmul(kv_ps[:, fc:fc + 1],
                                 wkTb[:, 128 * fc:128 * (fc + 1)], ctb,
                                 start=False, stop=True, skip_group_check=True)
            for fc in range(NFC):
                nc.tensor.matmul(kv_ps[:, NFC + fc:NFC + fc + 1],
                                 wvTa[:, 128 * fc:128 * (fc + 1)], cta,
                                 start=True, stop=False, skip_group_check=True)
                nc.tensor.matmul(kv_ps[:, NFC + fc:NFC + fc + 1],
                                 wvTb[:, 128 * fc:128 * (fc + 1)], ctb,
                                 start=False, stop=True, skip_group_check=True)
            S[k]["kv_ps"] = kv_ps

        def stage_kv_post(k):
            kv_ps = S[k]["kv_ps"]
            kbs = bpool.tile([128, NFC], f32, tag="kbs", name=f"kbs{k[0]}")
            nc.vector.tensor_add(kbs, kv_ps[:, 0:NFC], bks_pc)
            vbp = bpool.tile([128, NFC], f32, tag="vbp", name=f"vbp{k[0]}")
            nc.vector.tensor_add(vbp, kv_ps[:, NFC:2 * NFC], bv_pc)
            S[k]["kbs"], S[k]["vbp"] = kbs, vbp

        def stage_coef_pool(k):
            kbs, vbp = S[k]["kbs"], S[k]["vbp"]
            kpow = bpool.tile([128, NFC, 4], f32, tag="kpow", name=f"kp{k[0]}")
            nc.vector.memset(kpow[:, :, 0:1], 1.0)
            nc.vector.tensor_copy(kpow[:, :, 1:2],
                                  kbs.rearrange("p (f o) -> p f o", o=1))
            nc.vector.tensor_mul(kpow[:, :, 2], kbs, kbs)
            nc.vector.tensor_mul(kpow[:, :, 3], kpow[:, :, 2], kbs)
            kpown = bpool.tile([128, NFC, 4], f32, tag="kpown", name=f"kn{k[0]}")
            vb_b4 = bass.AP(tensor=vbp.tensor, offset=vbp.offset,
                            ap=[list(vbp.ap[0]), list(vbp.ap[1]), [0, 4]])
            nc.vector.tensor_mul(kpown, kpow, vb_b4)
            S[k]["kpow"], S[k]["kpown"] = kpow, kpown

        def stage_coef_q(k):
            """coefpair layout: [pn_0..pn_3 | pd_0..pd_3] (1/j! via factrow)"""
            kpow, kpown = S[k]["kpow"], S[k]["kpown"]
            q_ps = ps_sm.tile([1, 2, 4 * NFC], f32, tag="sm", name=f"q{k[0]}")
            nc.tensor.matmul(q_ps[:, 0, :], ones_col,
                             kpown.rearrange("p f j -> p (f j)"),
                             start=True, stop=True, skip_group_check=True)
            nc.tensor.matmul(q_ps[:, 1, :], ones_col,
                             kpow.rearrange("p f j -> p (f j)"),
                             start=True, stop=True, skip_group_check=True)
            # q layout: [T_0..T_3 | U_0..U_3] after f-reduce and 1/j! factors;
            # then the cubic Taylor of w = T(s)/U(s) itself:
            #   c0 = T0/U0, c1 = (T1-c0 U1)/U0, c2 = (T2-c0 U2-c1 U1)/U0,
            #   c3 = (T3-c0 U3-c1 U2-c2 U1)/U0
            coefraw = bpool.tile([1, 8], f32, tag="coefraw", name=f"cr{k[0]}")
            for c in range(2):
                qv = q_ps[:, c, :].rearrange("p (f j) -> p j f", f=NFC)
                nc.vector.tensor_reduce(coefraw[0:1, 4 * c:4 * (c + 1)],
                                        qv, axis=mybir.AxisListType.X,
                                        op=Alu.add)
            tu = bpool.tile([1, 8], f32, tag="tu", name=f"tu{k[0]}")
            nc.vector.tensor_mul(tu, coefraw, factrow)
            cw = bpool.tile([1, 4], f32, tag="cw", name=f"cw{k[0]}")
            sc = bpool.tile([1, 8], f32, tag="scr", name=f"scr{k[0]}")
            r0 = sc[0:1, 0:1]
            with nc.allow_low_precision(reason="scalar chain; 2e-2 budget"):
                nc.vector.reciprocal(r0, tu[0:1, 4:5])
                nc.vector.tensor_mul(cw[0:1, 0:1], tu[0:1, 0:1], r0)
                for j in range(1, 4):
                    acc_s = sc[0:1, j:j + 1]
                    nc.vector.tensor_mul(acc_s, cw[0:1, 0:1],
                                         tu[0:1, 4 + j:5 + j])
                    for i in range(1, j):
                        t_s = sc[0:1, 4 + i:5 + i]
                        nc.vector.tensor_mul(t_s, cw[0:1, i:i + 1],
                                             tu[0:1, 4 + j - i:5 + j - i])
                        nc.vector.tensor_add(acc_s, acc_s, t_s)
                    nc.vector.tensor_sub(acc_s, tu[0:1, j:j + 1], acc_s)
                    nc.vector.tensor_mul(cw[0:1, j:j + 1], acc_s, r0)
            cbc_ps = ps_sm.tile([128, 4], f32, tag="sm", name=f"cbc{k[0]}")
            nc.tensor.matmul(cbc_ps, ones_row_f, cw, start=True, stop=True)
            cbc = bpool.tile([128, 4], f32, tag="cbc", name=f"cbc{k[0]}")
            nc.vector.tensor_copy(cbc, cbc_ps)
            S[k]["cbc"] = cbc

        def stage_eval(k):
            s1p, s2p, s3p = S[k]["spows"]
            cbc = S[k]["cbc"]
            w_part = bpool.tile([128, 8], bf16, tag="w_part", name=f"wpt{k[0]}")
            t2 = bpool.tile([128, 8], bf16, tag="t2", name=f"t2{k[0]}")
            t3 = bpool.tile([128, 8], bf16, tag="t3", name=f"t3{k[0]}")
            with nc.allow_low_precision(reason="w bf16; 2e-2 budget"):
                nc.vector.tensor_scalar(w_part, s1p, cbc[:, 1:2], cbc[:, 0:1],
                                        op0=Alu.mult, op1=Alu.add)
                nc.vector.tensor_scalar(t2, s2p, cbc[:, 2:3], None, op0=Alu.mult)
                nc.vector.tensor_scalar(t3, s3p, cbc[:, 3:4], None, op0=Alu.mult)
                nc.vector.tensor_add(w_part, w_part, t2)
                nc.vector.tensor_add(w_part, w_part, t3)
            S[k]["w_part"] = w_part

        def stage_w_bcast(k, eng):
            """wrep[:, 128*nb+q] = w_part[q, nb] via zero-stride stationary
            (w column replicated over M) against the identity moving."""
            w_part = S[k]["w_part"]
            wrep_sb = bpool.tile([128, N], bf16, tag="wrep_sb",
                                 name=f"wrep{k[0]}")
            for h in range(2):
                hs = slice(512 * h, 512 * (h + 1))
                wrep_ps = ps_sm.tile([128, 512], f32, tag="sm",
                                     name=f"wrep{k[0]}{h}")
                for i in range(4):
                    nb = 4 * h + i
                    col = w_part[:, nb:nb + 1]
                    wb = bass.AP(tensor=col.tensor, offset=col.offset,
                                 ap=[list(col.ap[0]), [0, 128]])
                    nc.tensor.matmul(wrep_ps[:, 128 * i:128 * (i + 1)],
                                     wb, eye128,
                                     start=True, stop=True,
                                     skip_group_check=True)
                if eng == "act":
                    nc.scalar.copy(wrep_sb[:, hs], wrep_ps)
                else:
                    nc.vector.tensor_copy(wrep_sb[:, hs], wrep_ps)
            S[k]["wrep_sb"] = wrep_sb

        def stage_yout(k):
            xt, wrep_sb = S[k]["xt"], S[k]["wrep_sb"]
            o_sb = opool.tile([128, NCH, N], bf16, tag="o", name=f"o{k[0]}")
            for ch in range(NCH):
                y_sb = ypool.tile([128, N], bf16, tag="y", name=f"y{k[0]}{ch}")
                nc.gpsimd.tensor_scalar(y_sb, wrep_sb, wors_pc[:, ch:ch + 1],
                                        bo_pc[:, ch:ch + 1], op0=Alu.mult,
                                        op1=Alu.add)
                nc.gpsimd.tensor_add(o_sb[:, ch, :], xt[:, ch, :], y_sb)
            S[k]["o_sb"] = o_sb

        def stage_store(k):
            if store_out:
                nc.scalar.dma_start(
                    out_d[k[0]].rearrange("(a p) n -> p a n", p=128),
                    S[k]["o_sb"])

        stage_load(0, 0)
        stage_load(1, 0)
        for rep_i in range(reps):
            p = rep_i & 1
            k0, k1 = (0, p), (1, p)
            if rep_i + 1 < reps:
                stage_load(0, rep_i + 1)
                stage_load(1, rep_i + 1)
            stage_stats_a(k0)
            stage_stats_b(k0)
            stage_stats_a(k1)
            stage_smv_mm(k0)
            stage_s(k0)
            stage_kv(k0)
            stage_kv_post(k0)
            stage_coef_pool(k0)
            stage_coef_q(k0)
            stage_stats_b(k1)
            stage_eval(k0)
            stage_w_bcast(k0, "act")
            stage_smv_mm(k1)
            stage_s(k1)
            stage_kv(k1)
            stage_kv_post(k1)
            stage_coef_pool(k1)
            stage_coef_q(k1)
            stage_yout(k0)
            stage_eval(k1)
            stage_w_bcast(k1, "act")
            stage_store(k0)
            stage_yout(k1)
            stage_store(k1)

    if legalize:
        _legalize_sync(nc, mybir)
    return nc


def _indicators():
    ind128 = np.zeros((128, 8), np.float32)
    indT8 = np.zeros((8, 128), np.float32)
    for g in range(8):
        ind128[16 * g:16 * g + 16, g] = 1.0 / 16.0
        indT8[g, 16 * g:16 * g + 16] = 1.0
    return ind128, indT8


def _host_prep(inputs):
    import ml_dtypes
    bf = ml_dtypes.bfloat16
    f = {k: np.ascontiguousarray(np.asarray(v, dtype=np.float32))
         for k, v in inputs.items()}
    x = f["x"].reshape(B, C, N).astype(bf)
    cond = f["condition"].astype(bf)
    colsum = f["wq"].sum(axis=0)                       # [C]
    wqg = (colsum * f["gamma"]).astype(np.float32)
    wqb = np.array([f["bq"].sum() + (colsum * f["beta"]).sum()], np.float32)
    wo_rs = f["wo"].sum(axis=1).astype(np.float32)     # [C]
    wkT = np.ascontiguousarray((f["wk"] * SCALE).T).astype(bf)   # [T,F]
    wvT = np.ascontiguousarray(f["wv"].T).astype(bf)
    bks = (f["bk"] * SCALE).astype(np.float32)
    ind128, indT8 = _indicators()
    import ml_dtypes
    eye128 = np.eye(128, dtype=np.float32).astype(ml_dtypes.bfloat16)
    common = {
        "wqg": wqg, "wqb": wqb, "wo_rs": wo_rs, "bo": f["bo"],
        "bks": bks, "bv": f["bv"], "wkT": wkT, "wvT": wvT,
        "ind128": ind128, "indT8": indT8, "eye128": eye128,
    }
    return x, cond, common


def kernel(**inputs):
    from concourse.bass_utils import run_bass_kernel_spmd

    if "nc" not in _CACHE:
        _CACHE["nc"] = _build()
    nc = _CACHE["nc"]

    x, cond, common = _host_prep(inputs)
    in_maps = []
    for i in range(NCORES):
        in_maps.append({
            "x_sh": np.ascontiguousarray(x[BPC * i:BPC * (i + 1)]),
            "cond_sh": np.ascontiguousarray(cond[BPC * i:BPC * (i + 1)]),
            **common,
        })

    res = run_bass_kernel_spmd(nc, in_maps, core_ids=list(range(NCORES)))
    _CACHE["last_results"] = res
    out = np.concatenate([r["out"] for r in res.results], axis=0)
    return out.reshape(B, C, HW, HW).astype(np.float32)


# revision 52
# speedup vs baseline: 1.1237x; 1.1237x over previous
"""Trainium2 Bass kernel for nn_AttnBlock (B=16, C=512, H=W=32, T=180, G=32).

Math: the module broadcasts the text condition across channels, so k/v rows are
identical for every channel and the whole attention block collapses to rank-1:

  per batch b:
    group-norm stats over x[b]:   mu_g, rstd_g (32 groups of 16 ch x 1024 pix)
    wqg[c] = (sum_o wq[o,c]) * gamma[c];  a[c] = wqg[c]*rstd_{g(c)}
    s'[n]  = sum_c a[c]*x[c,n] + const_b
    kb[f]  = SCALE*(wk @ cond_b + bk) ;  vb[f] = wv @ cond_b + bv
    w[n]   = (sum_f vb[f]*e^{kb[f] s'[n]}) / (sum_f e^{kb[f] s'[n]})
    out[c,n] = x[c,n] + wo_rowsum[c]*w[n] + bo[c]

and since |kb*s'| stays small, w(s') is replaced by its cubic Taylor series
w ~= c0 + c1 s' + c2 s'^2 + c3 s'^3 whose coefficients come from the moment
sums T_j = sum_f vb kb^j/j!, U_j = sum_f kb^j/j! (truncation error ~2e-4,
well under the 2e-2 gate; bf16 staging dominates at ~6e-3).

Sharding: data-parallel over batch, 2 batches per core, 8 cores, no
collectives. x/out/cond ride HBM as bf16 (halves DMA, 2x DVE modes); k/v
matvecs and all moment/broadcast reductions run on PE against
host-transposed bf16 weights; weight-derived scalars are host-folded.
PSUM: 4 s-accumulator banks + 4 small-ring banks. Engine APs must start at
partition 0/32/64/96 - all row intermediates ([1,N] tiles) sit at p0.
"""
import numpy as np
from contextlib import ExitStack

B, C, HW, N, T = 16, 512, 32, 1024, 180
F = 1024                      # in_features == H*W
G = 32                        # groups; 16 channels per group
NCORES, BPC = 8, 2            # cores, batches per core
NCH = C // 128                # 4 channel chunks
NFC = F // 128                # 8 feature chunks
EPS = 1e-6
SCALE = float(C) ** -0.5

_CACHE = {}


def _legalize_sync(nc, mybir):
    """This walrus build accepts at most one sync-wait command per
    instruction; hoist extra waits onto preceding same-engine NOPs."""
    k = 0
    for fn in nc.m.functions:
        for blk in fn.blocks:
            new = []
            for ins in blk.instructions:
                si = ins.sync_info
                if si is not None and si.on_wait is not None and len(si.on_wait) > 1:
                    for w in list(si.on_wait[:-1]):
                        nop = mybir.InstNoOp(name=f"syncsplit-{k}", ins=[], outs=[])
                        k += 1
                        nop.engine = ins.engine
                        nop.sync_info = mybir.SyncInfo(on_wait=[w], on_update=[])
                        new.append(nop)
                    ins.sync_info = mybir.SyncInfo(
                        on_wait=[si.on_wait[-1]],
                        on_update=list(si.on_update or []))
                new.append(ins)
            blk.instructions[:] = new


def _build(reps=1, legalize=True, store_out=True, load_x=True, xbufs=4):
    import concourse.bass as bass
    import concourse.mybir as mybir
    import concourse.tile as tile

    f32 = mybir.dt.float32
    bf16 = mybir.dt.bfloat16
    Act = mybir.ActivationFunctionType
    Alu = mybir.AluOpType

    nc = bass.Bass()

    x_d = nc.dram_tensor("x_sh", [BPC, C, N], bf16, kind="ExternalInput")
    cond_d = nc.dram_tensor("cond_sh", [BPC, T], bf16, kind="ExternalInput")
    wqg_d = nc.dram_tensor("wqg", [C], f32, kind="ExternalInput")
    wors_d = nc.dram_tensor("wo_rs", [C], f32, kind="ExternalInput")
    bo_d = nc.dram_tensor("bo", [C], f32, kind="ExternalInput")
    bks_d = nc.dram_tensor("bks", [F], f32, kind="ExternalInput")
    bv_d = nc.dram_tensor("bv", [F], f32, kind="ExternalInput")
    wqb_d = nc.dram_tensor("wqb", [1], f32, kind="ExternalInput")
    wkT_d = nc.dram_tensor("wkT", [T, F], bf16, kind="ExternalInput")
    wvT_d = nc.dram_tensor("wvT", [T, F], bf16, kind="ExternalInput")
    ind128_d = nc.dram_tensor("ind128", [128, 8], f32, kind="ExternalInput")
    eye_d = nc.dram_tensor("eye128", [128, 128], bf16, kind="ExternalInput")
    indT8_d = nc.dram_tensor("indT8", [8, 128], f32, kind="ExternalInput")
    out_d = nc.dram_tensor("out", [BPC, C, N], bf16, kind="ExternalOutput")

    with tile.TileContext(nc) as tc, ExitStack() as ctx:
        singles = ctx.enter_context(tc.tile_pool(name="singles", bufs=1))
        xpool = ctx.enter_context(tc.tile_pool(name="xpool", bufs=xbufs))
        ypool = ctx.enter_context(tc.tile_pool(name="ypool", bufs=4))
        opool = ctx.enter_context(tc.tile_pool(name="opool", bufs=3))
        bpool = ctx.enter_context(tc.tile_pool(name="bpool", bufs=2))
        ps_sm = ctx.enter_context(tc.tile_pool(name="ps_sm", bufs=8, space="PSUM"))

        # constants + ACT exp-table preload first (ACT ring is in-order)
        ones_col = singles.tile([128, 1], f32)
        nc.vector.memset(ones_col, 1.0)
        ones_row_b = singles.tile([1, 128], bf16)
        nc.vector.memset(ones_row_b, 1.0)
        ones_row_f = singles.tile([1, 128], f32)
        nc.vector.memset(ones_row_f, 1.0)
        ones_row_n = singles.tile([1, N], bf16)
        nc.vector.memset(ones_row_n, 1.0)
        factrow = singles.tile([1, 8], f32)
        nc.vector.memset(factrow, 1.0)
        nc.vector.memset(factrow[0:1, 2:3], 0.5)
        nc.vector.memset(factrow[0:1, 3:4], 1.0 / 6.0)
        nc.vector.memset(factrow[0:1, 6:7], 0.5)
        nc.vector.memset(factrow[0:1, 7:8], 1.0 / 6.0)
        eps8 = singles.tile([8, 1], f32)
        nc.vector.memset(eps8, EPS)
        tl = singles.tile([1, 1], f32)
        nc.scalar.activation(tl, eps8[0:1, 0:1], Act.Exp)

        # ---------------- prologue loads (ACT ring; idle during body) -------
        xts, condTs = [], []
        xt0 = xpool.tile([128, NCH, N], bf16, tag="xt", name="xt0")
        nc.sync.dma_start(xt0, x_d[0].rearrange("(a p) n -> p a n", p=128))
        xt1 = xpool.tile([128, NCH, N], bf16, tag="xt", name="xt1")
        nc.sync.dma_start(xt1, x_d[1].rearrange("(a p) n -> p a n", p=128))
        xts.append(xt0)
        xts.append(xt1)
        for b in range(BPC):
            cta = bpool.tile([128, 1], bf16, tag="cta", name=f"cta{b}")
            nc.sync.dma_start(cta, cond_d[b, 0:128].rearrange("(p o) -> p o", o=1))
            ctb = bpool.tile([52, 1], bf16, tag="ctb", name=f"ctb{b}")
            nc.sync.dma_start(ctb, cond_d[b, 128:180].rearrange("(p o) -> p o", o=1))
            condTs.append((cta, ctb))
        wkTa = singles.tile([128, F], bf16)
        nc.scalar.dma_start(wkTa, wkT_d[0:128, :])
        wkTb = singles.tile([52, F], bf16)
        nc.scalar.dma_start(wkTb, wkT_d[128:180, :])
        wvTa = singles.tile([128, F], bf16)
        nc.scalar.dma_start(wvTa, wvT_d[0:128, :])
        wvTb = singles.tile([52, F], bf16)
        nc.scalar.dma_start(wvTb, wvT_d[128:180, :])
        ind128 = singles.tile([128, 8], f32)
        nc.scalar.dma_start(ind128, ind128_d[:, :])
        indT8 = singles.tile([8, 128], f32)
        nc.scalar.dma_start(indT8, indT8_d[:, :])
        wqg_pc = singles.tile([128, NCH], f32)
        nc.scalar.dma_start(wqg_pc, wqg_d[:].rearrange("(a p) -> p a", p=128))
        wors_pc = singles.tile([128, NCH], f32)
        nc.scalar.dma_start(wors_pc, wors_d[:].rearrange("(a p) -> p a", p=128))
        bo_pc = singles.tile([128, NCH], f32)
        nc.scalar.dma_start(bo_pc, bo_d[:].rearrange("(a p) -> p a", p=128))
        bks_pc = singles.tile([128, NFC], f32)
        nc.scalar.dma_start(bks_pc, bks_d[:].rearrange("(a p) -> p a", p=128))
        bv_pc = singles.tile([128, NFC], f32)
        nc.scalar.dma_start(bv_pc, bv_d[:].rearrange("(a p) -> p a", p=128))
        wqb_sb = singles.tile([1, 1], f32)
        nc.scalar.dma_start(wqb_sb, wqb_d[:].rearrange("(p o) -> p o", o=1))
        eye128 = singles.tile([128, 128], bf16)
        nc.scalar.dma_start(eye128, eye_d[:, :])

        # ---------------- per-batch stages (software-pipelined emission) ----
        # state keyed by (batch, rep parity) so rep r+1's front half can be
        # emitted while rep r's tail still reads its own tiles
        S = {}

        def stage_load(b, rep_i):
            k = (b, rep_i & 1)
            if rep_i == 0 or not load_x:
                if rep_i == 0:
                    S[k] = {"xt": xts[b], "condT": condTs[b]}
                else:
                    S[k] = {"xt": S[(b, (rep_i - 1) & 1)]["xt"],
                            "condT": S[(b, (rep_i - 1) & 1)]["condT"]}
            else:
                xt = xpool.tile([128, NCH, N], bf16, tag="xt", name=f"xtr{b}")
                nc.sync.dma_start(xt, x_d[b].rearrange("(a p) n -> p a n", p=128))
                cta = bpool.tile([128, 1], bf16, tag="cta", name=f"ctar{b}")
                nc.scalar.dma_start(cta,
                                    cond_d[b, 0:128].rearrange("(p o) -> p o", o=1))
                ctb = bpool.tile([52, 1], bf16, tag="ctb", name=f"ctbr{b}")
                nc.scalar.dma_start(ctb,
                                    cond_d[b, 128:180].rearrange("(p o) -> p o", o=1))
                S[k] = {"xt": xt, "condT": (cta, ctb)}

        def stage_stats_a(k):
            xt = S[k]["xt"]
            mv2 = bpool.tile([128, NCH, 2], f32, tag="mv2", name=f"mv2_{k[0]}")
            mv = bpool.tile([128, NCH, 2], f32, tag="mv", name=f"mv_{k[0]}")
            for ch in range(NCH):
                st = bpool.tile([128, 2, 6], f32, tag="st", name=f"st{k[0]}{ch}")
                nc.vector.bn_stats(st[:, 0, :], xt[:, ch, 0:512])
                nc.vector.bn_stats(st[:, 1, :], xt[:, ch, 512:1024])
                nc.vector.bn_aggr(mv[:, ch, :], st)
            msq = bpool.tile([128, NCH], f32, tag="msq", name=f"msq{k[0]}")
            nc.vector.tensor_mul(msq, mv[:, :, 0], mv[:, :, 0])
            nc.vector.tensor_copy(mv2[:, :, 0], mv[:, :, 0])
            nc.vector.tensor_add(mv2[:, :, 1], mv[:, :, 1], msq)
            S[k]["mv2"] = mv2

        def stage_stats_b(k):
            mv2 = S[k]["mv2"]
            gstat_ps = ps_sm.tile([8, NCH, 2], f32, tag="sm", name=f"gst{k[0]}")
            for ch in range(NCH):
                nc.tensor.matmul(gstat_ps[:, ch, :], ind128, mv2[:, ch, :],
                                 start=True, stop=True)
            gsb = bpool.tile([8, NCH, 2], f32, tag="gsb", name=f"gsb{k[0]}")
            nc.vector.tensor_copy(gsb, gstat_ps)
            msqg = bpool.tile([8, NCH], f32, tag="msqg", name=f"msqg{k[0]}")
            nc.vector.tensor_mul(msqg, gsb[:, :, 0], gsb[:, :, 0])
            varg = bpool.tile([8, NCH], f32, tag="varg", name=f"varg{k[0]}")
            nc.vector.tensor_sub(varg, gsb[:, :, 1], msqg)
            lnv = bpool.tile([8, NCH], f32, tag="lnv", name=f"lnv{k[0]}")
            nc.scalar.activation(lnv, varg, Act.Ln, bias=eps8[:, 0:1])
            rm = bpool.tile([8, 2, NCH], f32, tag="rm", name=f"rm{k[0]}")
            nc.scalar.activation(rm[:, 0, :], lnv, Act.Exp, scale=-0.5)
            nc.vector.tensor_mul(rm[:, 1, :], gsb[:, :, 0], rm[:, 0, :])
            rep_ps = ps_sm.tile([128, 2 * NCH], f32, tag="sm", name=f"rep{k[0]}")
            nc.tensor.matmul(rep_ps, indT8, rm.rearrange("g a c -> g (a c)"),
                             start=True, stop=True)
            rep3 = rep_ps.rearrange("p (a c) -> p a c", a=2)
            a_all = bpool.tile([128, NCH], bf16, tag="a_all", name=f"a_all{k[0]}")
            nc.vector.tensor_mul(a_all, wqg_pc, rep3[:, 0, :])
            wm_all = bpool.tile([128, NCH], f32, tag="wm_all", name=f"wm{k[0]}")
            nc.vector.tensor_mul(wm_all, wqg_pc, rep3[:, 1, :])
            S[k]["a_all"], S[k]["wm_all"] = a_all, wm_all

        def stage_smv_mm(k):
            """s in partition-major layout: sp[p, nb] = s[128*nb + p]."""
            a_all, wm_all, xt = S[k]["a_all"], S[k]["wm_all"], S[k]["xt"]
            sp_ps = ps_sm.tile([128, 8], f32, tag="sm", name=f"sp{k[0]}")
            wms_ps = ps_sm.tile([1, 1], f32, tag="sm", name=f"wms{k[0]}")
            for nb in range(8):
                for ch in range(NCH):
                    nc.tensor.matmul(
                        sp_ps[:, nb:nb + 1],
                        xt[:, ch, 128 * nb:128 * (nb + 1)],
                        a_all[:, ch:ch + 1],
                        start=(ch == 0), stop=(ch == NCH - 1),
                        skip_group_check=True)
            for ch in range(NCH):
                nc.tensor.matmul(wms_ps, wm_all[:, ch:ch + 1], ones_col,
                                 start=(ch == 0), stop=(ch == NCH - 1))
            S[k]["sp_ps"], S[k]["wms_ps"] = sp_ps, wms_ps

        def stage_s(k):
            """partition-major powers sp^1..sp^3, each [128, 8]."""
            sp_ps, wms_ps = S[k]["sp_ps"], S[k]["wms_ps"]
            constb = bpool.tile([1, 1], f32, tag="constb", name=f"cb{k[0]}")
            nc.vector.tensor_sub(constb, wqb_sb, wms_ps)
            cb_ps = ps_sm.tile([128, 1], f32, tag="sm", name=f"cbb{k[0]}")
            nc.tensor.matmul(cb_ps, ones_row_f, constb, start=True, stop=True)
            cb_sb = bpool.tile([128, 1], f32, tag="cb_sb", name=f"cbs{k[0]}")
            nc.vector.tensor_copy(cb_sb, cb_ps)
            s1p = bpool.tile([128, 8], bf16, tag="s1p", name=f"s1p{k[0]}")
            with nc.allow_low_precision(reason="s bf16; 2e-2 budget"):
                nc.vector.tensor_scalar(s1p, sp_ps, cb_sb, None, op0=Alu.add)
            s2p = bpool.tile([128, 8], bf16, tag="s2p", name=f"s2p{k[0]}")
            nc.vector.tensor_mul(s2p, s1p, s1p)
            s3p = bpool.tile([128, 8], bf16, tag="s3p", name=f"s3p{k[0]}")
            nc.vector.tensor_mul(s3p, s2p, s1p)
            S[k]["spows"] = (s1p, s2p, s3p)

        def stage_kv(k):
            cta, ctb = S[k]["condT"]
            kv_ps = ps_sm.tile([128, 2 * NFC], f32, tag="sm", name=f"kv{k[0]}")
            for fc in range(NFC):
                nc.tensor.matmul(kv_ps[:, fc:fc + 1],
                                 wkTa[:, 128 * fc:128 * (fc + 1)], cta,
                                 start=True, stop=False, skip_group_check=True)
                nc.tensor.matmul(kv_ps[:, fc:fc + 1],
                                 wkTb[:, 128 * fc:128 * (fc + 1)], ctb,
                                 start=False, stop=True, skip_group_check=True)
            for fc in range(NFC):
                nc.tensor.matmul(kv_ps[:, NFC + fc:NFC + fc + 1],
                                 wvTa[:, 128 * fc:128 * (fc + 1)], cta,
                                 start=True, stop=False, skip_group_check=True)
                nc.tensor.matmul(kv_ps[:, NFC + fc:NFC + fc + 1],
                                 wvTb[:, 128 * fc:128 * (fc + 1)], ctb,
                                 start=False, stop=True, skip_group_check=True)
            S[k]["kv_ps"] = kv_ps

        def stage_kv_post(k):
            kv_ps = S[k]["kv_ps"]
            kbs = bpool.tile([128, NFC], f32, tag="kbs", name=f"kbs{k[0]}")
            nc.vector.tensor_add(kbs, kv_ps[:, 0:NFC], bks_pc)
            vbp = bpool.tile([128, NFC], f32, tag="vbp", name=f"vbp{k[0]}")
            nc.vector.tensor_add(vbp, kv_ps[:, NFC:2 * NFC], bv_pc)
            S[k]["kbs"], S[k]["vbp"] = kbs, vbp

        def stage_coef_pool(k):
            kbs, vbp = S[k]["kbs"], S[k]["vbp"]
            kpow = bpool.tile([128, NFC, 4], f32, tag="kpow", name=f"kp{k[0]}")
            nc.vector.memset(kpow[:, :, 0:1], 1.0)
            nc.vector.tensor_copy(kpow[:, :, 1:2],
                                  kbs.rearrange("p (f o) -> p f o", o=1))
            nc.vector.tensor_mul(kpow[:, :, 2], kbs, kbs)
            nc.vector.tensor_mul(kpow[:, :, 3], kpow[:, :, 2], kbs)
            kpown = bpool.tile([128, NFC, 4], f32, tag="kpown", name=f"kn{k[0]}")
            vb_b4 = bass.AP(tensor=vbp.tensor, offset=vbp.offset,
                            ap=[list(vbp.ap[0]), list(vbp.ap[1]), [0, 4]])
            nc.vector.tensor_mul(kpown, kpow, vb_b4)
            S[k]["kpow"], S[k]["kpown"] = kpow, kpown

        def stage_coef_q(k):
            """coefpair layout: [pn_0..pn_3 | pd_0..pd_3] (1/j! via factrow)"""
            kpow, kpown = S[k]["kpow"], S[k]["kpown"]
            q_ps = ps_sm.tile([1, 2, 4 * NFC], f32, tag="sm", name=f"q{k[0]}")
            nc.tensor.matmul(q_ps[:, 0, :], ones_col,
                             kpown.rearrange("p f j -> p (f j)"),
                             start=True, stop=True, skip_group_check=True)
            nc.tensor.matmul(q_ps[:, 1, :], ones_col,
                             kpow.rearrange("p f j -> p (f j)"),
                             start=True, stop=True, skip_group_check=True)
            # q layout: [T_0..T_3 | U_0..U_3] after f-reduce and 1/j! factors;
            # then the cubic Taylor of w = T(s)/U(s) itself:
            #   c0 = T0/U0, c1 = (T1-c0 U1)/U0, c2 = (T2-c0 U2-c1 U1)/U0,
            #   c3 = (T3-c0 U3-c1 U2-c2 U1)/U0
            coefraw = bpool.tile([1, 8], f32, tag="coefraw", name=f"cr{k[0]}")
            for c in range(2):
                qv = q_ps[:, c, :].rearrange("p (f j) -> p j f", f=NFC)
                nc.vector.tensor_reduce(coefraw[0:1, 4 * c:4 * (c + 1)],
                                        qv, axis=mybir.AxisListType.X,
                                        op=Alu.add)
            tu = bpool.tile([1, 8], f32, tag="tu", name=f"tu{k[0]}")
            nc.vector.tensor_mul(tu, coefraw, factrow)
            cw = bpool.tile([1, 4], f32, tag="cw", name=f"cw{k[0]}")
            sc = bpool.tile([1, 8], f32, tag="scr", name=f"scr{k[0]}")
            r0 = sc[0:1, 0:1]
            with nc.allow_low_precision(reason="scalar chain; 2e-2 budget"):
                nc.vector.reciprocal(r0, tu[0:1, 4:5])
                nc.vector.tensor_mul(cw[0:1, 0:1], tu[0:1, 0:1], r0)
                for j in range(1, 4):
                    acc_s = sc[0:1, j:j + 1]
                    nc.vector.tensor_mul(acc_s, cw[0:1, 0:1],
                                         tu[0:1, 4 + j:5 + j])
                    for i in range(1, j):
                        t_s = sc[0:1, 4 + i:5 + i]
                        nc.vector.tensor_mul(t_s, cw[0:1, i:i + 1],
                                             tu[0:1, 4 + j - i:5 + j - i])
                        nc.vector.tensor_add(acc_s, acc_s, t_s)
                    nc.vector.tensor_sub(acc_s, tu[0:1, j:j + 1], acc_s)
                    nc.vector.tensor_mul(cw[0:1, j:j + 1], acc_s, r0)
            cbc_ps = ps_sm.tile([128, 4], f32, tag="sm", name=f"cbc{k[0]}")
            nc.tensor.matmul(cbc_ps, ones_row_f, cw, start=True, stop=True)
            cbc = bpool.tile([128, 4], f32, tag="cbc", name=f"cbc{k[0]}")
            nc.vector.tensor_copy(cbc, cbc_ps)
            S[k]["cbc"] = cbc

        def stage_eval(k):
            s1p, s2p, s3p = S[k]["spows"]
            cbc = S[k]["cbc"]
            w_part = bpool.tile([128, 8], bf16, tag="w_part", name=f"wpt{k[0]}")
            t2 = bpool.tile([128, 8], bf16, tag="t2", name=f"t2{k[0]}")
            t3 = bpool.tile([128, 8], bf16, tag="t3", name=f"t3{k[0]}")
            with nc.allow_low_precision(reason="w bf16; 2e-2 budget"):
                nc.vector.tensor_scalar(w_part, s1p, cbc[:, 1:2], cbc[:, 0:1],
                                        op0=Alu.mult, op1=Alu.add)
                nc.vector.tensor_scalar(t2, s2p, cbc[:, 2:3], None, op0=Alu.mult)
                nc.vector.tensor_scalar(t3, s3p, cbc[:, 3:4], None, op0=Alu.mult)
                nc.vector.tensor_add(w_part, w_part, t2)
                nc.vector.tensor_add(w_part, w_part, t3)
            S[k]["w_part"] = w_part

        def stage_w_bcast(k, eng):
            """wrep[:, 128*nb+q] = w_part[q, nb] via zero-stride stationary
            (w column replicated over M) against the identity moving."""
            w_part = S[k]["w_part"]
            wrep_sb = bpool.tile([128, N], bf16, tag="wrep_sb",
                                 name=f"wrep{k[0]}")
            for h in range(2):
                hs = slice(512 * h, 512 * (h + 1))
                wrep_ps = ps_sm.tile([128, 512], f32, tag="sm",
                                     name=f"wrep{k[0]}{h}")
                for i in range(4):
                    nb = 4 * h + i
                    col = w_part[:, nb:nb + 1]
                    wb = bass.AP(tensor=col.tensor, offset=col.offset,
                                 ap=[list(col.ap[0]), [0, 128]])
                    nc.tensor.matmul(wrep_ps[:, 128 * i:128 * (i + 1)],
                                     wb, eye128,
                                     start=True, stop=True,
                                     skip_group_check=True)
                if eng == "act":
                    nc.scalar.copy(wrep_sb[:, hs], wrep_ps)
                else:
                    nc.vector.tensor_copy(wrep_sb[:, hs], wrep_ps)
            S[k]["wrep_sb"] = wrep_sb

        def stage_yout(k):
            xt, wrep_sb = S[k]["xt"], S[k]["wrep_sb"]
            o_sb = opool.tile([128, NCH, N], bf16, tag="o", name=f"o{k[0]}")
            for ch in range(NCH):
                y_sb = ypool.tile([128, N], bf16, tag="y", name=f"y{k[0]}{ch}")
                nc.gpsimd.tensor_scalar(y_sb, wrep_sb, wors_pc[:, ch:ch + 1],
                                        bo_pc[:, ch:ch + 1], op0=Alu.mult,
                                        op1=Alu.add)
                nc.gpsimd.tensor_add(o_sb[:, ch, :], xt[:, ch, :], y_sb)
            if store_out:
                nc.sync.dma_start(
                    out_d[k[0]].rearrange("(a p) n -> p a n", p=128), o_sb)

        stage_load(0, 0)
        stage_load(1, 0)
        for rep_i in range(reps):
            p = rep_i & 1
            k0, k1 = (0, p), (1, p)
            if rep_i + 1 < reps:
                stage_load(0, rep_i + 1)
                stage_load(1, rep_i + 1)
            stage_stats_a(k0)
            stage_stats_b(k0)
            stage_stats_a(k1)
            stage_smv_mm(k0)
            stage_s(k0)
            stage_kv(k0)
            stage_kv_post(k0)
            stage_coef_pool(k0)
            stage_coef_q(k0)
            stage_stats_b(k1)
            stage_eval(k0)
            stage_w_bcast(k0, "act")
            stage_smv_mm(k1)
            stage_s(k1)
            stage_kv(k1)
            stage_kv_post(k1)
            stage_coef_pool(k1)
            stage_coef_q(k1)
            stage_yout(k0)
            stage_eval(k1)
            stage_w_bcast(k1, "act")
            stage_yout(k1)

    if legalize:
        _legalize_sync(nc, mybir)
    return nc


def _indicators():
    ind128 = np.zeros((128, 8), np.float32)
    indT8 = np.zeros((8, 128), np.float32)
    for g in range(8):
        ind128[16 * g:16 * g + 16, g] = 1.0 / 16.0
        indT8[g, 16 * g:16 * g + 16] = 1.0
    return ind128, indT8


def _host_prep(inputs):
    import ml_dtypes
    bf = ml_dtypes.bfloat16
    f = {k: np.ascontiguousarray(np.asarray(v, dtype=np.float32))
         for k, v in inputs.items()}
    x = f["x"].reshape(B, C, N).astype(bf)
    cond = f["condition"].astype(bf)
    colsum = f["wq"].sum(axis=0)                       # [C]
    wqg = (colsum * f["gamma"]).astype(np.float32)
    wqb = np.array([f["bq"].sum() + (colsum * f["beta"]).sum()], np.float32)
    wo_rs = f["wo"].sum(axis=1).astype(np.float32)     # [C]
    wkT = np.ascontiguousarray((f["wk"] * SCALE).T).astype(bf)   # [T,F]
    wvT = np.ascontiguousarray(f["wv"].T).astype(bf)
    bks = (f["bk"] * SCALE).astype(np.float32)
    ind128, indT8 = _indicators()
    import ml_dtypes
    eye128 = np.eye(128, dtype=np.float32).astype(ml_dtypes.bfloat16)
    common = {
        "wqg": wqg, "wqb": wqb, "wo_rs": wo_rs, "bo": f["bo"],
        "bks": bks, "bv": f["bv"], "wkT": wkT, "wvT": wvT,
        "ind128": ind128, "indT8": indT8, "eye128": eye128,
    }
    return x, cond, common


def kernel(**inputs):
    from concourse.bass_utils import run_bass_kernel_spmd

    if "nc" not in _CACHE:
        _CACHE["nc"] = _build()
    nc = _CACHE["nc"]

    x, cond, common = _host_prep(inputs)
    in_maps = []
    for i in range(NCORES):
        in_maps.append({
            "x_sh": np.ascontiguousarray(x[BPC * i:BPC * (i + 1)]),
            "cond_sh": np.ascontiguousarray(cond[BPC * i:BPC * (i + 1)]),
            **common,
        })

    res = run_bass_kernel_spmd(nc, in_maps, core_ids=list(range(NCORES)))
    _CACHE["last_results"] = res
    out = np.concatenate([r["out"] for r in res.results], axis=0)
    return out.reshape(B, C, HW, HW).astype(np.float32)
